# revision 44
# baseline (speedup 1.0000x reference)
"""Approximate rank pooling (segment-reduce) on 8 TRN2 NeuronCores.

Strategy: the per-frame weight w[t] depends only on vidids (tiny), so it is
computed on the host exactly as the reference does. The heavy part -- the
weighted segment sum over x [2048, 3*128*128] -- runs on device as a matmul:
each core c gets an equal slice of 256 frames, x_c [256, 49152], plus a
host-built weight matrix W_c [256, NV] whose row i has w[global_t] at column
(vidids[global_t] - v_lo_c) and zeros elsewhere.  The core computes
out_c = W_c^T @ x_c with TensorEngine accumulating over the two 128-frame
K-chunks in PSUM.  The host then scatters/adds the per-core partial outputs
into the full [64, 3, 128, 128] result (a video straddling a core boundary
simply gets contributions from both cores).

Precision: the rel-err budget is 2e-2.  x is fed as fp8 e3m4 (quantization
RMS ~1.34%, measured 1.341% end-to-end on the actual inputs), W as fp16,
PSUM accumulates in fp32, out is stored as fp16 (adds ~0.01%).  Per-core
HBM reads drop from 50.3 MB (f32) to 12.6 MB.

TensorE: with M=nv~12 only 12/128 PE columns are used, so NSTRIPS column
tiles (tile_position=(0,32j)) run concurrent matmuls, cutting the
moving-stream floor ~NSTRIPS-fold; the kernel is then DMA-bound.  Each
iteration covers NSTRIPS*PW contiguous columns with ONE [128, NSTRIPS*PW]
load per K-chunk, so load completion order matches consumption order, and
all 16 loads are issued upfront (xpool bufs=16) so the 16 HW DMA queues
never starve on tile recycling.

PSUM evacuation: ACT does one [64+nv, PW] copy per iteration (it has
dedicated SBUF ports; DVE copies enter 2-port mode and block the gpsimd
SWDGE stores), then three per-strip stores ride the gpsimd SOFTWARE DGE
(HW-DGE-ring stores would park semaphore waits in the striped HW queues
and block loads behind them).  Tail special-casing: iteration NITER-2's
stores use the scalar HW ring (loads have drained by then, and this
frees gpsimd), and the final iteration copies on the idle DVE so the
copy starts the moment its last matmul retires.
"""

import numpy as np

T, C, H, W = 2048, 3, 128, 128
D = C * H * W              # 49152
NCORES = 8
TL = T // NCORES           # 256 frames per core
KP = 128                   # K chunk = SBUF partition count
NK = TL // KP              # 2 K-chunks
NSTRIPS = 3                # PE column tiles running concurrently
SUB = 512                  # one fp32 PSUM bank
PW = 2048                  # max columns per strip per iteration (4 banks)
# Tapered iteration widths: the loads all stream at the same total rate,
# but a narrow final iteration leaves only a tiny MM->copy->store chain
# after the last byte arrives.
PWS = [2048] * 7 + [1536, 512]
NITER = len(PWS)
assert sum(PWS) * NSTRIPS == D

X_DTYPE = "float8e3"       # e3m4: 1 B/elem, ~1.34% RMS quantization error
W_DTYPE = "float16"
OUT_DTYPE = "float16"


def _frame_weights(vid: np.ndarray, nvids: int) -> np.ndarray:
    """Replicates the reference weight math in numpy (float32)."""
    T_ = vid.shape[0]
    counts = np.bincount(vid, minlength=nvids).astype(np.int64)
    starts = np.cumsum(counts) - counts
    N = counts[vid]                                    # [T] segment size
    t = np.arange(T_, dtype=np.int64) - starts[vid] + 1  # [T] 1-based rank
    Hh = np.zeros(T_ + 1, dtype=np.float32)
    Hh[1:] = np.cumsum(
        (1.0 / np.arange(1, T_ + 1, dtype=np.float32)).astype(np.float32),
        dtype=np.float32,
    )
    poly = (N * (N + 1) - t * (t - 1) - N * (N - t + 1)).astype(np.float32)
    w = poly - (Hh[N] - Hh[t - 1])
    return np.where(N == 1, np.float32(1.0), w).astype(np.float32)


def _build_nc(nv: int):
    import concourse.bacc as bacc
    import concourse.tile as tile
    from concourse import mybir

    assert nv <= 32, f"col-tiling needs nv<=32, got {nv}"
    xdt = getattr(mybir.dt, X_DTYPE)
    wdt = getattr(mybir.dt, W_DTYPE)
    odt = getattr(mybir.dt, OUT_DTYPE)
    f32 = mybir.dt.float32

    nc = bacc.Bacc("TRN2", target_bir_lowering=False, debug=False)
    x = nc.dram_tensor("x", [TL, D], xdt, kind="ExternalInput").ap()
    wt = nc.dram_tensor("wt", [TL, nv], wdt, kind="ExternalInput").ap()
    out = nc.dram_tensor("out", [nv, D], odt, kind="ExternalOutput").ap()

    np_copy = 32 * (NSTRIPS - 1) + nv
    col0s = [sum(PWS[:i]) * NSTRIPS for i in range(NITER)]

    def store(ti, ot, engs):
        """Per-strip stores from an evacuated SBUF tile.

        Mid-kernel stores use the gpsimd SWDGE (HW-DGE-ring stores would
        park semaphore waits ahead of pending loads); the final
        iterations' stores may ride the sync/scalar HW rings, which are
        empty once all loads have drained.
        """
        pw = PWS[ti]
        for j in range(NSTRIPS):
            oc = col0s[ti] + j * pw
            getattr(nc, engs[j % len(engs)]).dma_start(
                out[:, oc:oc + pw], ot[32 * j:32 * j + nv, :pw]
            )

    with tile.TileContext(nc) as tc:
        with (
            tc.tile_pool(name="wpool", bufs=1) as wpool,
            tc.tile_pool(name="xpool", bufs=2 * NITER) as xpool,
            tc.tile_pool(name="opool", bufs=4) as opool,
            tc.tile_pool(name="psum", bufs=2, space="PSUM") as ppool,
        ):
            wtiles = []
            for k in range(NK):
                wtile = wpool.tile([KP, nv], wdt, tag=f"w{k}")
                nc.sync.dma_start(wtile[:], wt[k * KP:(k + 1) * KP, :])
                wtiles.append(wtile)

            xts = []
            for ti in range(NITER):
                lw = NSTRIPS * PWS[ti]
                per = []
                for k in range(NK):
                    xt = xpool.tile([KP, NSTRIPS * PW], xdt,
                                    name="xt", tag="xt")
                    # All loads on the SP HWDGE ring (striped across all
                    # 16 SDMA engines).  Splitting across both rings
                    # measured a slightly better best-case but a wider
                    # spread; the single ring is the tightest distribution.
                    nc.sync.dma_start(
                        xt[:, :lw], x[k * KP:(k + 1) * KP,
                                      col0s[ti]:col0s[ti] + lw]
                    )
                    per.append(xt)
                xts.append(per)

            for ti in range(NITER):
                pw = PWS[ti]
                pt = ppool.tile([KP, PW], f32, name="pt", tag="pt")
                for k in range(NK):
                    for s in range(pw // SUB):
                        for j in range(NSTRIPS):
                            nc.tensor.matmul(
                                pt[32 * j:32 * j + nv,
                                   s * SUB:(s + 1) * SUB],
                                wtiles[k][:],
                                xts[ti][k][:, j * pw + s * SUB:
                                           j * pw + (s + 1) * SUB],
                                start=(k == 0),
                                stop=(k == NK - 1),
                                tile_position=(0, 32 * j),
                                skip_group_check=True,
                            )
                ot = opool.tile([32 * NSTRIPS, PW], odt,
                                name="ot", tag="ot")
                if ti == NITER - 1:
                    # Final (narrow) iteration: copy on the idle DVE so it
                    # starts the moment the last matmul wave retires (the
                    # scalar FIFO is still pushing iteration NITER-2's
                    # ring descriptors); stores split across the idle
                    # gpsimd SWDGE and the drained sync HW ring.
                    nc.vector.tensor_copy(ot[:np_copy, :pw],
                                          pt[:np_copy, :pw])
                    store(ti, ot, engs=["gpsimd", "sync", "gpsimd"])
                elif ti == NITER - 2:
                    # Loads have drained; these stores ride the scalar HW
                    # ring so gpsimd is free for the final iteration.
                    nc.scalar.copy(ot[:np_copy, :pw], pt[:np_copy, :pw])
                    store(ti, ot, engs=["scalar"])
                else:
                    nc.scalar.copy(ot[:np_copy, :pw], pt[:np_copy, :pw])
                    store(ti, ot, engs=["gpsimd"])

    nc.compile()
    return nc


def _run(x, vidids, nvids, trace=False, trace_cores=None):
    import ml_dtypes
    from concourse.bass_utils import run_bass_kernel_spmd

    x = np.ascontiguousarray(np.asarray(x, dtype=np.float32))
    vid = np.asarray(vidids).astype(np.int64).ravel()
    nv_total = int(nvids)
    assert x.shape == (T, C, H, W) and vid.shape == (T,)

    w = _frame_weights(vid, nv_total)
    xq = x.reshape(T, D).astype(ml_dtypes.float8_e3m4)

    v_lo, nv_local = [], []
    for c in range(NCORES):
        lo, hi = c * TL, (c + 1) * TL
        v_lo.append(int(vid[lo]))
        nv_local.append(int(vid[hi - 1]) - int(vid[lo]) + 1)
    NV = max(nv_local)

    in_maps = []
    rows = np.arange(TL)
    for c in range(NCORES):
        lo = c * TL
        Wc = np.zeros((TL, NV), dtype=np.float32)
        Wc[rows, vid[lo:lo + TL] - v_lo[c]] = w[lo:lo + TL]
        in_maps.append({"x": xq[lo:lo + TL], "wt": Wc.astype(np.float16)})

    nc = _build_nc(NV)
    res = run_bass_kernel_spmd(
        nc, in_maps, list(range(NCORES)), trace=trace, trace_cores=trace_cores
    )

    outf = np.zeros((nv_total, D), dtype=np.float32)
    for c in range(NCORES):
        part = np.asarray(res.results[c]["out"]).astype(np.float32)
        n = min(NV, nv_total - v_lo[c])
        outf[v_lo[c]:v_lo[c] + n] += part[:n]
    return outf.reshape(nv_total, C, H, W), res


def kernel(x, vidids, nvids):
    out, _ = _run(x, vidids, nvids)
    return out
